# revision 12
# baseline (speedup 1.0000x reference)
"""LoRA-MoE fused linear (grouped ragged GEMM) on 8 TRN2 NeuronCores.

Strategy: expert-parallel with host-side LoRA weight merging. Since
y = x @ w_base + (x @ w_a) @ w_b * s  ==  x @ (w_base + s * w_a @ w_b)
per expert, the rank-16 LoRA paths (which waste the 128-wide PE array)
are folded into the base weights on the host (fp32), leaving a pure
ragged GEMM on device.

The 64 experts are LPT-bin-packed onto 8 cores (8 experts/core) to
balance token counts. Tokens are pre-sorted by expert, so each expert's
rows are a contiguous slice of x. Host packs, per core:
  xt : bf16 [2048, CAP]            x^T columns grouped by expert slot
  w  : bf16 [8, 6, 128, 16, 128]   [slot, nt, kp, kc, n] merged weights
  yt : bf16 [768, CAP]             output (y^T), unpacked on host
Kernel computes yt[n, c] = sum_k W[k,n] x[c,k] per slot with the token
dim on the matmul free axis (m <= 512 per PSUM bank), k-chunked by 128.
Weights stream per (slot, nt) in 0.5MB tiles so the first matmuls can
start ~1us into the slot's DMA instead of waiting for the full 5MB.
Per-slot capacities are compile-time constants derived from m_sizes
(compiled at call time, cached).
"""

import sys

if "/opt/trn_rl_repo" not in sys.path:
    sys.path.insert(0, "/opt/trn_rl_repo")

import numpy as np
import ml_dtypes

T, IN, OUT, E, R = 32768, 2048, 768, 64, 16
SCALING = 2.0
NC_CORES = 8
EPC = E // NC_CORES  # experts per core
KC = IN // 128       # 16 contraction chunks
NT = OUT // 128      # 6 output-feature tiles
MAX_N = 512          # PSUM bank limit (fp32 columns)
MT_N = 288           # m-tile target: HW streams bf16 ~2 cols/cycle below ~300
                     # (measured 0.31 ns/col at m=128-286 vs 0.43 at m=512)
BF16 = ml_dtypes.bfloat16

_cache: dict = {}


def _split_sync_waits(nc, max_waits=1):
    """walrus in this container rejects >1 sync-wait on an instruction;
    split extras onto preceding NoOps on the same engine."""
    import concourse.mybir as mybir

    n_split = 0
    for fn in nc.m.functions:
        for bb in fn.blocks:
            new_insts = []
            for ins in bb.instructions:
                si = getattr(ins, "sync_info", None)
                waits = list(si.on_wait) if si is not None and si.on_wait else []
                if len(waits) > max_waits:
                    k = 0
                    while len(waits) - k > max_waits:
                        chunk = waits[k : k + max_waits]
                        k += max_waits
                        nop = mybir.InstNoOp(
                            name=nc.get_next_instruction_name(),
                            ins=[],
                            outs=[],
                            sync_info=mybir.SyncInfo(on_wait=chunk, on_update=[]),
                        )
                        nop.engine = ins.engine
                        new_insts.append(nop)
                        n_split += 1
                    si.on_wait = waits[k:]
                new_insts.append(ins)
            bb.instructions[:] = new_insts
    return n_split


def _dedupe_ldweights(nc):
    """Remove InstLdweights whose stationary AP matches the previous one,
    with only weight-preserving instructions in between on the PE stream
    (consecutive matmuls over the two m-tiles of a slot share weights).
    Sync info of a removed Ldweights is preserved on a NoOp."""
    import concourse.mybir as mybir

    n_drop = 0
    for fn in nc.m.functions:
        for bb in fn.blocks:
            last_key = None
            new_insts = []
            for ins in bb.instructions:
                tn = type(ins).__name__
                if tn == "InstLdweights":
                    key = str(ins.ins[0])
                    if key == last_key:
                        si = getattr(ins, "sync_info", None)
                        if si is not None and (
                            (si.on_wait and len(si.on_wait))
                            or (si.on_update and len(si.on_update))
                        ):
                            nop = mybir.InstNoOp(
                                name=nc.get_next_instruction_name(),
                                ins=[],
                                outs=[],
                                sync_info=si,
                            )
                            nop.engine = ins.engine
                            new_insts.append(nop)
                        n_drop += 1
                        continue
                    last_key = key
                elif tn in (
                    "InstMatmult",
                    "InstNoOp",
                    "InstEventSemaphore",
                    "InstRegisterMove",
                ):
                    pass  # these leave loaded weights intact
                else:
                    last_key = None
                new_insts.append(ins)
            bb.instructions[:] = new_insts
    return n_drop


def _mtiles(cap):
    """Split a slot's column span into even tiles of <= MT_N."""
    nt = -(-cap // MT_N)
    base = -(-(-(-cap // nt)) // 4) * 4  # ceil(cap/nt) rounded up to mult of 4
    tiles = []
    c0 = 0
    for i in range(nt):
        ml = min(base, cap - c0)
        if ml <= 0:
            break
        tiles.append((c0, ml))
        c0 += ml
    return tiles


def _build(caps, rep=1):
    import concourse.bass as bass
    import concourse.mybir as mybir
    import concourse.tile as tile

    CAP = int(sum(caps))
    nc = bass.Bass()
    xt_h = nc.declare_dram_parameter("xt", [IN, CAP], mybir.dt.bfloat16, isOutput=False)
    w_h = nc.declare_dram_parameter(
        "w", [EPC, NT, 128, KC, 128], mybir.dt.bfloat16, isOutput=False
    )
    yt_h = nc.declare_dram_parameter("yt", [OUT, CAP], mybir.dt.bfloat16, isOutput=True)

    xt = xt_h[:].rearrange("(kc kp) c -> kp kc c", kp=128)  # [128, KC, CAP]
    yt = yt_h[:].rearrange("(nt np) c -> np nt c", np=128)  # [128, NT, CAP]

    with tile.TileContext(nc) as tc:
        with (
            tc.tile_pool(name="xtp", bufs=2) as xtp,
            tc.tile_pool(name="wp", bufs=2 * NT) as wp,
            tc.tile_pool(name="outp", bufs=2) as outp,
            tc.tile_pool(name="psp", bufs=8, space="PSUM") as psp,
        ):
          for _rep in range(rep):
            col0 = 0
            for s, cap in enumerate(caps):
                cap = int(cap)
                if cap == 0:
                    continue
                xts = xtp.tile([128, KC, cap], mybir.dt.bfloat16, tag="xts")
                wts = [
                    wp.tile([128, KC, 128], mybir.dt.bfloat16, tag="wts", name="wts")
                    for _ in range(NT)
                ]
                # interleave x (kc-chunks) and w (nt-chunks) loads so the
                # matmul stream can chase the DMA stream within the slot
                nc.sync.dma_start(out=xts[:, 0, :], in_=xt[:, 0, col0 : col0 + cap])
                nc.sync.dma_start(out=wts[0][:], in_=w_h[s, 0])
                wi = 1
                for kc in range(1, KC):
                    nc.sync.dma_start(
                        out=xts[:, kc, :], in_=xt[:, kc, col0 : col0 + cap]
                    )
                    if kc % 3 == 0 and wi < NT:
                        nc.sync.dma_start(out=wts[wi][:], in_=w_h[s, wi])
                        wi += 1
                while wi < NT:
                    nc.sync.dma_start(out=wts[wi][:], in_=w_h[s, wi])
                    wi += 1

                outs = outp.tile([128, NT, cap], mybir.dt.bfloat16, tag="outs")

                # process all m-tiles of a slot per (nt, kc) so consecutive
                # matmuls share the stationary operand; _dedupe_ldweights then
                # drops the redundant Ldweights
                mts = _mtiles(cap)
                for nt in range(NT):
                    pss = [
                        psp.tile([128, ml], mybir.dt.float32, tag="ps", name="ps")
                        for _, ml in mts
                    ]
                    for kc in range(KC):
                        for (c0, ml), ps in zip(mts, pss):
                            nc.tensor.matmul(
                                ps[:],
                                wts[nt][:, kc, :],
                                xts[:, kc, c0 : c0 + ml],
                                start=(kc == 0),
                                stop=(kc == KC - 1),
                            )
                    for (c0, ml), ps in zip(mts, pss):
                        nc.vector.tensor_copy(outs[:, nt, c0 : c0 + ml], ps[:])
                # per-mtile output DMA shrinks the kernel tail; issued on
                # the Activation HWDGE queue so stores never delay the
                # load prefetch stream on the SP queue
                for c0, ml in mts:
                    nc.scalar.dma_start(
                        out=yt[:, :, col0 + c0 : col0 + c0 + ml],
                        in_=outs[:, :, c0 : c0 + ml],
                    )
                col0 += cap

    # NOTE: _dedupe_ldweights measured SLOWER on HW (per-MM Ldweights enables
    # background weight-buffer overlap) — intentionally not called.
    _split_sync_waits(nc)
    return nc


def _plan(m_sizes):
    """LPT-balanced assignment of experts to cores; per-slot capacities."""
    m = np.asarray(m_sizes, dtype=np.int64)
    offs = np.zeros(E + 1, dtype=np.int64)
    np.cumsum(np.maximum(m, 0), out=offs[1:])
    # effective sizes clipped to the token count
    starts = np.minimum(offs[:-1], T)
    ends = np.minimum(offs[1:], T)
    eff = ends - starts

    order = np.argsort(-eff, kind="stable")
    load = np.zeros(NC_CORES, dtype=np.int64)
    slots = [[] for _ in range(NC_CORES)]
    for e in order:
        cands = [c for c in range(NC_CORES) if len(slots[c]) < EPC]
        c = min(cands, key=lambda i: (load[i], i))
        slots[c].append(int(e))
        load[c] += eff[e]
    # slots[c] is descending in eff by construction
    caps = tuple(
        int(-(-max(int(eff[slots[c][s]]) for c in range(NC_CORES)) // 4) * 4)
        for s in range(EPC)
    )
    return slots, caps, starts, eff


def _pack(x, m_sizes, w_base, w_a, w_b, plan):
    """Host-side: merge LoRA into base weights, shard + lay out per core."""
    slots, caps, starts, eff = plan
    CAP = int(sum(caps))
    colstart = np.zeros(EPC + 1, dtype=np.int64)
    np.cumsum(np.asarray(caps), out=colstart[1:])

    x = np.ascontiguousarray(np.asarray(x), dtype=np.float32)
    w_base = np.asarray(w_base, dtype=np.float32)
    w_a = np.asarray(w_a, dtype=np.float32)
    w_b = np.asarray(w_b, dtype=np.float32)

    # merged weights: w_base + s * w_a @ w_b  (exact algebra of the
    # reference; fp32 on host, then one bf16 cast like the base path)
    w_eff = w_base + SCALING * np.matmul(w_a, w_b)
    web = w_eff.astype(BF16)
    xb = x.astype(BF16)

    in_maps = []
    for c in range(NC_CORES):
        exps = slots[c]
        xt = np.zeros((IN, CAP), dtype=BF16)
        for s, e in enumerate(exps):
            n = int(eff[e])
            if n:
                xt[:, colstart[s] : colstart[s] + n] = xb[
                    starts[e] : starts[e] + n
                ].T
        # [s, nt, kp, kc, n] so each (slot, nt) weight tile is one DMA with
        # 4KB contiguous per-partition lines
        wc = (
            web[exps]
            .reshape(EPC, KC, 128, NT, 128)
            .transpose(0, 3, 2, 1, 4)
        )
        in_maps.append({"xt": xt, "w": np.ascontiguousarray(wc)})
    return in_maps


def _unpack(results, plan):
    slots, caps, starts, eff = plan
    colstart = np.zeros(EPC + 1, dtype=np.int64)
    np.cumsum(np.asarray(caps), out=colstart[1:])
    out = np.zeros((T, OUT), dtype=np.float32)
    for c in range(NC_CORES):
        yt = results[c]["yt"]
        for s, e in enumerate(slots[c]):
            n = int(eff[e])
            if n:
                out[starts[e] : starts[e] + n] = (
                    yt[:, colstart[s] : colstart[s] + n].T.astype(np.float32)
                )
    return out


def kernel(x, m_sizes, w_base, w_a, w_b):
    plan = _plan(m_sizes)
    caps = plan[1]
    if caps not in _cache:
        _cache[caps] = _build(caps)
    nc = _cache[caps]

    in_maps = _pack(x, m_sizes, w_base, w_a, w_b, plan)

    from concourse.bass_utils import run_bass_kernel_spmd

    res = run_bass_kernel_spmd(nc, in_maps, core_ids=list(range(NC_CORES)))
    return _unpack(res.results, plan)
